# revision 8
# baseline (speedup 1.0000x reference)
"""Causal multi-head attention (B=2, S=2048, D=2048, 32 heads x 64) for 8
Trainium2 NeuronCores.

Sharding: data parallel on batch (2 groups of 4 cores) x tensor parallel on
heads (4 groups of 8 heads each). Each core computes q/k/v projections for
its head group, RoPE, causal attention with sigmoid-gated values, and a
partial o-projection; the host sums the 4 partials per batch (the
"all-reduce" of the o-projection) and adds the output bias + gate-mean
constant.

Design (evolved from the ~481us bf16 kernel):
- q/k projections and the AV matmul run in fp8 DoubleRow perf mode (two
  128-deep k-tiles contracted per instruction, ~1.5x bf16 throughput at
  512-wide moving operands). x is DMA'd once in bf16 and down-converted
  to fp8 on the idle gpsimd engine; q/k weights are scaled x16 on the
  host so W*16 sits in e4m3's good range. The 1/16 compensation folds
  into the RoPE cos/sin tables; the k-side 1/8 softmax scale folds into
  the exp activation's input scale.
- The V projection stays bf16: v/gate noise feeds the output directly
  (no softmax averaging), and the fp8 v-path alone costs ~1.5e-2 of the
  2e-2 error budget.
- Gates are stored hi-lo in one 128-wide fp8 block per head:
  [64 x tanh-hi | ones | 63 x 16*(tanh - hi)]. The DoubleRow AV matmul
  emits hi rows, the softmax denominator, and a 16x-scaled residual in
  one pass (output partitions 65 -> 128 are otherwise idle PE columns,
  so the residual is free); the normalize combines hi + lo/16, cutting
  gate quantization error ~20x. Dim 63 rides hi-only.
- at2 = exp(s/8 - 2) is written as fp8e5 (e5m2) straight from the
  activation engine: scores reach 8.2, and e5m2's 57344 max makes
  overflow impossible where e4m3's 240 NaN'd; at2 precision cancels in
  the softmax ratio for concentrated rows, so the mantissa loss is
  cheap.
- Scores contract only 64 partitions (head dim) so fp8 buys nothing
  there; they stay bf16. Causal masking is a PE identity-matmul
  accumulation, width-restricted (with scores/exp/AV) to pair-aligned
  windows so fully-dead key-tile columns left of the diagonal are never
  touched.
- o-projection stays bf16 end to end (fp8 y/Wo alone measures 2.8e-2).
- PSUM pools stay open across phases; software-pipelined interleaving
  keeps the PE dense: attention for query blocks 0-1 is woven into the
  half-1 projection stream; o-projs of finished blocks spread through
  the next block's attention; AV matmuls trail their exp by two pairs.
- RoPE uses a host-side d-permutation (pairs (d, d+32) on adjacent
  partitions) so rotate-half is a within-quadrant stream_shuffle lane
  swap; weights, biases and cos/sin tables are permuted to match.
- Gate = tanh(v/2) = 2*sigmoid(v)-1: the 0.5 folds into Wo and the +0.5
  mean term becomes a host constant; tanh shares the exp activation
  table.
"""

import os

import numpy as np
import ml_dtypes

import concourse.bacc as bacc
import concourse.tile as tile
from concourse import mybir
from concourse.bass_utils import run_bass_kernel_spmd

B, S, D = 2, 2048, 2048
H_PER_CORE = 8          # heads per core
DH = 64                 # head dim
CW = 512                # per-core projection width = H_PER_CORE * DH
N_CORES = 8
KT = D // 128           # k-subtiles for the D-contraction
VST = 1024              # va tile-block stride (8 heads x 128)
WS = 16.0               # fp8 weight pre-scale

f32 = mybir.dt.float32
bf16 = mybir.dt.bfloat16
fp8 = mybir.dt.float8e4
fp8e5 = mybir.dt.float8e5
Act = mybir.ActivationFunctionType
Alu = mybir.AluOpType
DR = mybir.MatmulPerfMode.DoubleRow

TRACE = bool(int(os.environ.get("KERNEL_TRACE", "0")))
LAST_EXEC_NS = None
LAST_MEAN_NS = None

_SENT = object()
_SWAP_MASK = [i ^ 1 for i in range(32)]


def _build(WITH_BIAS=True):
    nc = bacc.Bacc("TRN2", target_bir_lowering=False, debug=False)

    x4 = nc.dram_tensor("x4", [4, 128, KT, 512], bf16, kind="ExternalInput")
    wq4 = nc.dram_tensor("wq4", [4, 128, KT, 128], fp8, kind="ExternalInput")
    wk4 = nc.dram_tensor("wk4", [4, 128, KT, 128], fp8, kind="ExternalInput")
    wv4 = nc.dram_tensor("wv4", [128, KT, CW], bf16, kind="ExternalInput")
    wo4 = nc.dram_tensor("wo4", [4, 128, 4, 512], bf16, kind="ExternalInput")
    bq = nc.dram_tensor("bq", [1, CW], bf16, kind="ExternalInput")
    bk = nc.dram_tensor("bk", [1, CW], bf16, kind="ExternalInput")
    bv = nc.dram_tensor("bv", [1, CW], bf16, kind="ExternalInput")
    ropec = nc.dram_tensor("ropec", [4, 128, 512], f32, kind="ExternalInput")
    ropes = nc.dram_tensor("ropes", [4, 128, 512], f32, kind="ExternalInput")
    masks = nc.dram_tensor("masks", [128, 4, 512], bf16, kind="ExternalInput")
    ident = nc.dram_tensor("ident", [128, 128], bf16, kind="ExternalInput")
    vinit = nc.dram_tensor("vinit", [128, 16 * VST], fp8, kind="ExternalInput")
    part = nc.dram_tensor("part", [S, D], bf16, kind="ExternalOutput")

    with tile.TileContext(nc) as tc:
        with (
            tc.tile_pool(name="p0", bufs=1) as p0,
            tc.tile_pool(name="pqk", bufs=1) as pqk,
            tc.tile_pool(name="py", bufs=1) as py,
            tc.tile_pool(name="pa", bufs=1) as pa,
            tc.tile_pool(name="paw", bufs=5) as paw,
            tc.tile_pool(name="prc", bufs=1) as prc,
            tc.tile_pool(name="prt", bufs=2) as prt,
            tc.tile_pool(name="prv", bufs=2) as prv,
            tc.tile_pool(name="pba", bufs=4) as pba,
            tc.tile_pool(name="pbs", bufs=2) as pbs,
            tc.tile_pool(name="pc", bufs=2) as pc,
            tc.tile_pool(name="pbo", bufs=2) as pbo,
        ):
            # persistent state
            qt_all = pqk.tile([128, 4, S], bf16, name="qt_all")
            kt_all = pqk.tile([128, 4, S], bf16, name="kt_all")
            qt = [qt_all[:, i, :] for i in range(4)]
            kt = [kt_all[:, i, :] for i in range(4)]
            # gated values: 16 seq-tiles x 8 heads x [64 hi | ones | 63 lo]
            va_all = p0.tile([128, 16 * VST], fp8, name="va_all")
            va4 = va_all[:].rearrange("p (k h d) -> p k h d", k=16, h=8)
            onesb = p0.tile([1, CW], bf16, name="onesb")
            bvt = p0.tile([1, CW], bf16, name="bvt")
            bqrow = p0.tile([1, CW], bf16, name="bqrow")
            bkrow = p0.tile([1, CW], bf16, name="bkrow")
            idt = p0.tile([128, 128], bf16, name="idt")
            maskt = p0.tile([128, 4, 512], bf16, name="maskt")
            wvf = p0.tile([128, KT, CW], bf16, name="wvf")
            ytr = [py.tile([128, S], bf16, name=f"ytr{i}") for i in range(4)]
            bneg = p0.tile([128, 1], f32, name="bneg")

            warm = p0.tile([1, 8], f32, name="warm")

            def load_xh(half, first_wch=None):
                xhv = pa.tile([128, 2, KT, 512], bf16, tag="xhv", name="xhv")
                xh8 = pa.tile([128, 2, KT, 512], fp8, tag="xh8", name="xh8")
                if first_wch is not None:
                    first_wch()
                for kg in range(4):
                    for qloc in range(2):
                        nc.sync.dma_start(
                            xhv[:, qloc, 4 * kg:4 * kg + 4, :],
                            x4[2 * half + qloc, :, 4 * kg:4 * kg + 4, :])
                        # fp8 shadow for the DoubleRow q/k matmuls, made on
                        # the otherwise-idle gpsimd engine
                        nc.gpsimd.tensor_copy(
                            xh8[:, qloc, 4 * kg:4 * kg + 4, :],
                            xhv[:, qloc, 4 * kg:4 * kg + 4, :])
                return xhv, xh8

            def load_rope(half):
                cosw = prc.tile([128, 2, 512], f32, tag="tblc", name="cosw")
                rsnw = prc.tile([128, 2, 512], f32, tag="tbls", name="rsnw")
                for qloc in range(2):
                    nc.sync.dma_start(cosw[:, qloc, :], ropec[2 * half + qloc])
                    nc.sync.dma_start(rsnw[:, qloc, :], ropes[2 * half + qloc])
                return cosw, rsnw

            def emit_consts():
                nc.sync.dma_start(idt[:], ident[:])
                nc.sync.dma_start(maskt[:], masks[:])
                nc.vector.memset(bneg[:], -2.0)
                if WITH_BIAS:
                    nc.vector.memset(onesb[:], 1.0)
                    nc.sync.dma_start(bvt[:], bv[:])
                    nc.sync.dma_start(bqrow[:], bq[:])
                    nc.sync.dma_start(bkrow[:], bk[:])

            def emit_vconsts():
                # needed only from the V-projection phase onward
                for kg in range(4):
                    nc.sync.dma_start(wvf[:, 4 * kg:4 * kg + 4, :],
                                      wv4[:, 4 * kg:4 * kg + 4, :])
                # va default 1.0 -> per-head ones column stays 1 (softmax
                # denominator rides the AV matmul)
                for vg in range(4):
                    nc.sync.dma_start(
                        va_all[:, vg * 4 * VST:(vg + 1) * 4 * VST],
                        vinit[:, vg * 4 * VST:(vg + 1) * 4 * VST])

            def load_wch(w3, mt):
                wch = paw.tile([128, KT, 128], fp8, tag="wch", name="wch")
                nc.sync.dma_start(wch[:], w3[mt])
                return wch

            def qk_unit(wch, dall, brow, mt, half, xh8, cosw, rsnw, pmain,
                        hook=None):
                ps = pmain.tile([128, 2, 512], f32, tag="psa", name="ps_a")
                for kp in range(KT // 2):
                    if kp == 4 and hook is not None:
                        hook()
                    for qloc in range(2):
                        # qloc inner: consecutive matmuls share the
                        # stationary weight pair
                        nc.tensor.matmul(
                            ps[:, qloc, :], wch[:, 2 * kp:2 * kp + 2, :],
                            xh8[:, qloc, 2 * kp:2 * kp + 2, :],
                            start=(kp == 0),
                            stop=(kp == KT // 2 - 1 and not WITH_BIAS),
                            perf_mode=DR,
                        )
                if WITH_BIAS:
                    for qloc in range(2):
                        nc.tensor.matmul(
                            ps[:, qloc, :],
                            brow[0:1, mt * 128:(mt + 1) * 128],
                            onesb[0:1, 0:512], start=False, stop=True,
                        )
                # RoPE with host-interleaved d-order (pairs (d, d+32) sit on
                # adjacent partitions; scores are invariant to the shared
                # q/k permutation): rotate-half becomes a within-quadrant
                # stream_shuffle lane swap, so every op is full-width and
                # partition-aligned. cos/sin tables carry the 1/WS fp8
                # compensation.
                d3 = dall[:, mt, half * 1024:(half + 1) * 1024
                          ].rearrange("p (a b) -> p a b", a=2)
                tmp = prt.tile([128, 2, 512], bf16, tag="tmp", name="tmp")
                tmp2 = prt.tile([128, 2, 512], bf16, tag="tmp2", name="tmp2")
                nc.vector.tensor_mul(tmp[:], ps[:], rsnw[:])
                nc.vector.stream_shuffle(tmp2[:], tmp[:], _SWAP_MASK)
                nc.vector.tensor_mul(d3[:], ps[:], cosw[:])
                nc.vector.tensor_add(d3[:], d3[:], tmp2[:])

            def v_unit(qloc, sp, half, xhv, pmain, hook=None):
                # two seq tiles (st = 2*sp, 2*sp+1) per PSUM pair; bf16
                # matmuls (fp8 v noise feeds the output un-averaged)
                qtr = 2 * half + qloc
                stg0 = qtr * 4 + 2 * sp
                psb = pmain.tile([128, 2, 512], f32, tag="psa", name="ps_v")
                for j in range(2):
                    if j == 1 and hook is not None:
                        hook()
                    st = 2 * sp + j
                    for k in range(KT):
                        nc.tensor.matmul(
                            psb[:, j, :],
                            xhv[:, qloc, k, st * 128:(st + 1) * 128],
                            wvf[:, k, :],
                            start=(k == 0),
                            stop=(k == KT - 1 and not WITH_BIAS),
                        )
                    if WITH_BIAS:
                        nc.tensor.matmul(
                            psb[:, j, :], onesb[0:1, 0:128], bvt[:],
                            start=False, stop=True,
                        )
                # gate = tanh(v/2) = 2*sigmoid(v)-1 (0.5 folded into Wo,
                # +0.5 mean term added on host). Full-precision tanh lands
                # in bf16; the fp8 block gets hi = fp8(g) and a 16x-scaled
                # residual lo (dims 0..62) for the hi-lo AV matmul.
                vab = va_all[:, VST * stg0:VST * (stg0 + 2)].rearrange(
                    "p (s h d) -> p s h d", s=2, h=8)
                tvt = prv.tile([128, 2, 8, 64], bf16, tag="tvt", name="tvt")
                nc.scalar.activation(
                    tvt[:], psb[:].rearrange("p a (h d) -> p a h d", h=8),
                    Act.Tanh, scale=0.5,
                )
                nc.vector.tensor_copy(vab[:, :, :, 0:64], tvt[:])
                # residual stored unscaled: e4m3 subnormals still cut gate
                # quantization ~17x, and the normalize needs no rescale.
                # Layout [64 hi | 63 lo | ones@127] keeps every engine
                # access on a 32-aligned partition base downstream.
                nc.vector.tensor_sub(vab[:, :, :, 64:127],
                                     tvt[:, :, :, 0:63],
                                     vab[:, :, :, 0:63])

            def att_head(qb, pi, hh, psco, psy):
                npair = 2 * qb + 2
                h = 2 * pi + hh
                lo, hi = hh * 64, (hh + 1) * 64
                yps = psy.tile([128, 512], f32, tag="yps", name="ps_y")

                def _w0(kp):
                    # pair-aligned causal window: key pair kp is fully dead
                    # for query columns < 256*(kp - 2*qb)
                    return 0 if kp < 2 * qb else 256 * (kp - 2 * qb)

                def _av(at2, kp):
                    w0 = _w0(kp)
                    nc.tensor.matmul(
                        yps[:, w0:], va4[:, 2 * kp:2 * kp + 2, h, :],
                        at2[:, :, w0:],
                        start=(kp == 0), stop=(kp == npair - 1),
                        perf_mode=DR,
                    )

                pipe = []
                for kp in range(npair):
                    w0 = _w0(kp)
                    ps2 = psco.tile([128, 2, 512], f32, tag="pss",
                                    name="ps_s")
                    for j in range(2):
                        k_i = 2 * kp + j
                        dt_i = k_i - 4 * qb
                        nc.tensor.matmul(
                            ps2[:, j, w0:],
                            kt[pi][lo:hi, k_i * 128:(k_i + 1) * 128],
                            qt[pi][lo:hi, qb * 512 + w0:(qb + 1) * 512],
                            start=True, stop=(dt_i < 0),
                        )
                        if dt_i >= 0:
                            wend = 128 * (dt_i + 1)
                            nc.tensor.matmul(
                                ps2[:, j, w0:wend], idt[:],
                                maskt[:, dt_i, w0:wend],
                                start=False, stop=True,
                            )
                    at2 = pba.tile([128, 2, 512], fp8e5, tag="at", name="at2")
                    # s/8 folds the softmax scale (k weights ship x16, not
                    # x2); e5m2 output cannot overflow (max score ~8.2)
                    nc.scalar.activation(at2[:, :, w0:], ps2[:, :, w0:],
                                         Act.Exp, bias=bneg[:], scale=0.125)
                    if len(pipe) == 2:
                        _av(*pipe.pop(0))
                        yield
                    pipe.append((at2, kp))
                for item in pipe:
                    _av(*item)
                    yield
                # engine partition bases must be 32-aligned and only one
                # TensorTensor input may read PSUM, so: pull quadrant 3 to
                # SBUF, extract the den row (partition 127) with a tiny
                # partition-shifting DMA on the Act queue, then combine
                # hi + lo in SBUF.
                dsb = pbs.tile([32, 512], f32, tag="dsb", name="dsb")
                nc.vector.tensor_copy(dsb[:], yps[96:128, :])
                den = pbs.tile([1, 512], f32, tag="den", name="den")
                nc.scalar.dma_start(den[:], dsb[31:32, :])
                rc = pbs.tile([1, 512], f32, tag="rc", name="rc")
                nc.vector.reciprocal_approx_fast(rc[:], den[:])
                s128 = pbs.tile([128, 512], f32, tag="s128", name="s128")
                nc.gpsimd.partition_broadcast(s128[:], rc[:])
                nc.vector.tensor_mul(
                    ytr[pi][lo:hi, qb * 512:(qb + 1) * 512],
                    yps[0:64, :], s128[lo:hi, :],
                )
                # tlo written at base `lo` so the SBUF-SBUF add below has
                # equal base partitions (a TensorTensor requirement)
                tlo = pbs.tile([128, 512], bf16, tag="tlo", name="tlo")
                nc.vector.tensor_mul(tlo[lo:lo + 64, :], yps[64:128, :],
                                     s128[64:128, :])
                nc.vector.tensor_add(
                    ytr[pi][lo:lo + 63, qb * 512:(qb + 1) * 512],
                    ytr[pi][lo:lo + 63, qb * 512:(qb + 1) * 512],
                    tlo[lo:lo + 63, :],
                )

            def att_qbs(qbs, psco, psy):
                for qb in qbs:
                    for pi in range(4):
                        for hh in range(2):
                            yield from att_head(qb, pi, hh, psco, psy)

            def oproj_gen(qb, pso):
                for nt in range(4):
                    woc = pc.tile([128, 4, 512], bf16, tag="woc", name="woc")
                    nc.sync.dma_start(woc[:], wo4[nt])
                    for sl in range(4):
                        st = 4 * qb + sl
                        ps = pso.tile([128, 512], f32, tag="pso", name="ps_o")
                        for kc in range(4):
                            nc.tensor.matmul(
                                ps[:], ytr[kc][:, st * 128:(st + 1) * 128],
                                woc[:, kc, :],
                                start=(kc == 0), stop=(kc == 3),
                            )
                        ostg = pbo.tile([128, 512], bf16, tag="ostg",
                                        name="ostg")
                        # DVE, not Act: staging on the scalar engine queues
                        # behind the exp backlog and strangles o-proj
                        nc.vector.tensor_copy(ostg[:], ps[:])
                        nc.sync.dma_start(
                            part[st * 128:(st + 1) * 128,
                                 nt * 512:(nt + 1) * 512],
                            ostg[:],
                        )
                        yield

            def drive(gen, filler, nf, npair):
                """Pull the attention generator, spreading `nf` filler steps
                proportionally across `npair` attention pairs."""
                n = 0
                pulled = 0
                for _ in gen:
                    n += 1
                    while pulled < n * nf // npair:
                        if next(filler, _SENT) is _SENT:
                            pulled = nf
                            break
                        pulled += 1
                for _ in filler:
                    pass

            wsets = ((wq4, qt_all, bqrow), (wk4, kt_all, bkrow))

            with (
                tc.tile_pool(name="pmain", bufs=2, space="PSUM") as pmain,
                tc.tile_pool(name="psco", bufs=1, space="PSUM") as psco,
                tc.tile_pool(name="psy1", bufs=2, space="PSUM") as psy1,
            ):
                # ------------- half 0: q/k/v projections -------------
                wchq = [None] * 5

                def _first_wch():
                    wchq[0] = load_wch(wq4, 0)

                nc.vector.memset(warm[:], 0.0)
                nc.scalar.activation(warm[:], warm[:], Act.Exp)
                xhv0, xh80 = load_xh(0, _first_wch)
                cosw0, rsnw0 = load_rope(0)
                for i in range(1, 4):
                    wchq[i] = load_wch(wq4, i)
                wchq[4] = load_wch(wk4, 0)
                emit_consts()
                for wi, (w3, dall, brow) in enumerate(wsets):
                    for mt in range(4):
                        pre = 4 * wi + mt
                        wch = wchq[pre] if pre < 5 else load_wch(w3, mt)
                        if pre == 2:
                            emit_vconsts()
                        qk_unit(wch, dall, brow, mt, 0, xh80, cosw0, rsnw0,
                                pmain)
                for qloc in range(2):
                    for sp in range(2):
                        v_unit(qloc, sp, 0, xhv0, pmain)

                # ---- half 1 interleaved with attention qb0/qb1 ----
                xhv1, xh81 = load_xh(1)
                cosw1, rsnw1 = load_rope(1)
                g01 = att_qbs((0, 1), psco, psy1)

                def take(n):
                    for _ in range(n):
                        if next(g01, _SENT) is _SENT:
                            return

                for w3, dall, brow in wsets:
                    for mt in range(4):
                        take(2)
                        qk_unit(load_wch(w3, mt), dall, brow, mt, 1, xh81,
                                cosw1, rsnw1, pmain, hook=lambda: take(2))
                for qloc in range(2):
                    for sp in range(2):
                        take(2)
                        v_unit(qloc, sp, 1, xhv1, pmain,
                               hook=lambda: take(2))
                for _ in g01:
                    pass

            # ---- attention qb2/qb3 with o-proj of finished blocks ----
            with (
                tc.tile_pool(name="pssB", bufs=2, space="PSUM") as pssB,
                tc.tile_pool(name="psyB", bufs=2, space="PSUM") as psyB,
                tc.tile_pool(name="pso", bufs=2, space="PSUM") as pso,
            ):
                import itertools
                f01 = itertools.chain(oproj_gen(0, pso), oproj_gen(1, pso))
                drive(att_qbs((2,), pssB, psyB),
                      itertools.islice(f01, 24), 24, 48)
                drive(att_qbs((3,), pssB, psyB),
                      itertools.chain(f01, oproj_gen(2, pso)), 24, 64)
                for _ in oproj_gen(3, pso):
                    pass

    nc.compile()
    return nc


def _rope_tables():
    half = DH // 2
    inv_freq = 1.0 / (10000.0 ** (np.arange(0, half, dtype=np.float32) / half))
    t = np.arange(S, dtype=np.float32)
    freqs = np.einsum("i,j->ij", t, inv_freq)            # [S, 32]
    emb = np.concatenate([freqs, freqs], axis=-1)        # [S, 64]
    cos = np.cos(emb).T.astype(np.float32)                        # [64, S]
    sin = np.sin(emb).T.astype(np.float32)
    rsin = np.concatenate([-sin[:32], sin[32:]], axis=0)
    return np.ascontiguousarray(np.concatenate([cos, rsin], axis=0))  # [128, S]


def _masks():
    j = np.arange(128)[:, None, None]
    dt = np.arange(4)[None, :, None]
    i = np.arange(512)[None, None, :]
    keep = (128 * dt + j) <= i
    return np.where(keep, 0.0, -1e30).astype(np.float32)  # [128, 4, 512]


def _bf(a):
    return np.ascontiguousarray(a).astype(ml_dtypes.bfloat16)


def _f8(a):
    return np.ascontiguousarray(a).astype(ml_dtypes.float8_e4m3)


def kernel(**inputs):
    global LAST_EXEC_NS
    x = np.asarray(inputs["x"], dtype=np.float32)
    Wq = np.asarray(inputs["Wq"], dtype=np.float32)
    Wk = np.asarray(inputs["Wk"], dtype=np.float32)
    Wv = np.asarray(inputs["Wv"], dtype=np.float32)
    Wo = np.asarray(inputs["Wo"], dtype=np.float32)
    bq = np.asarray(inputs["bq"], dtype=np.float32)
    bk = np.asarray(inputs["bk"], dtype=np.float32)
    bv = np.asarray(inputs["bv"], dtype=np.float32)
    bo = np.asarray(inputs["bo"], dtype=np.float32)

    ropeT = _rope_tables()
    masks = _masks()

    with_bias = any(float(np.abs(b).max()) > 0 for b in (bq, bk, bv))
    nc = _build(WITH_BIAS=with_bias)
    # d-order interleave: RoPE partner (d, d+32) -> adjacent partitions, so
    # rotate-half is a stream_shuffle lane swap. Scores are invariant to the
    # shared q/k permutation; W columns, biases and tables permute together.
    perm64 = np.empty(64, np.int64)
    perm64[0::2] = np.arange(32)          # new 2j   <- old j
    perm64[1::2] = np.arange(32) + 32     # new 2j+1 <- old j+32
    idx512 = (np.arange(0, 512, 64)[:, None] + perm64[None, :]).reshape(512)
    cos64 = ropeT[0:64][perm64]                       # [64, S] permuted
    cosT = np.concatenate([cos64, cos64], axis=0)     # [128, S] dup
    sin0 = -ropeT[64:96]                              # +sin rows (j<32)
    rs64 = np.empty((64, S), np.float32)
    rs64[0::2] = sin0                                 # new 2j   -> +sin_j
    rs64[1::2] = -sin0                                # new 2j+1 -> -sin_j
    rsT = np.concatenate([rs64, rs64], axis=0)        # [128, S]
    # 1/WS compensates the x16 fp8 weight pre-scale
    ropec = np.ascontiguousarray(
        (cosT / WS).reshape(128, 4, 512).transpose(1, 0, 2))
    ropes = np.ascontiguousarray(
        (rsT / WS).reshape(128, 4, 512).transpose(1, 0, 2))
    vinit = _f8(np.ones((128, 16 * VST), dtype=np.float32))
    ident = _bf(np.eye(128, dtype=np.float32))
    masks_b = _bf(masks)
    in_maps = []
    for c in range(N_CORES):
        b, g = c // 4, c % 4
        sl = slice(CW * g, CW * (g + 1))
        xT = x[b].T                                    # [D, S]
        x4 = _bf(xT.reshape(KT, 128, 4, 512).transpose(2, 1, 0, 3))
        wq4 = _f8((Wq[sl][idx512].T * WS)
                  .reshape(KT, 128, 4, 128).transpose(2, 1, 0, 3))
        # k ships x16 like q; the softmax 1/8 folds into the exp scale
        wk4 = _f8((Wk[sl][idx512].T * WS)
                  .reshape(KT, 128, 4, 128).transpose(2, 1, 0, 3))
        wv4 = _bf(Wv[sl].T.reshape(KT, 128, CW).transpose(1, 0, 2))
        wo4 = _bf(
            (0.5 * Wo[:, sl]).T.reshape(4, 128, 4, 512).transpose(2, 1, 0, 3))
        in_maps.append({
            "x4": x4,
            "wq4": wq4,
            "wk4": wk4,
            "wv4": wv4,
            "wo4": wo4,
            "bq": _bf((bq[sl][idx512] * WS).reshape(1, CW)),
            "bk": _bf((bk[sl][idx512] * WS).reshape(1, CW)),
            "bv": _bf(bv[sl].reshape(1, CW)),
            "ropec": ropec,
            "ropes": ropes,
            "vinit": vinit,
            "ident": ident,
            "masks": masks_b,
        })

    kwargs = {}
    if TRACE:
        kwargs = dict(trace=True, trace_cores=list(range(N_CORES)),
                      stitch_traces=False)
        tdir = os.environ.get("KERNEL_TRACE_DIR")
        if tdir:
            os.makedirs(tdir, exist_ok=True)
            kwargs["tmpdir"] = tdir
    global LAST_MEAN_NS
    r = run_bass_kernel_spmd(nc, in_maps, list(range(N_CORES)), **kwargs)
    LAST_EXEC_NS = r.exec_time_ns
    LAST_MEAN_NS = r.mean_exec_time_ns

    # host "all-reduce": sum the 4 head-group partials per batch, add the
    # output bias and the 0.5*rowsum(Wo) term from the centered gate.
    const = bo + 0.5 * Wo.sum(axis=1)
    out = np.empty((B, S, D), dtype=np.float32)
    for b in range(B):
        acc = r.results[4 * b]["part"].astype(np.float32).copy()
        for g in range(1, 4):
            acc += r.results[4 * b + g]["part"]
        out[b] = acc + const
    return out


# revision 10
# speedup vs baseline: 1.4574x; 1.4574x over previous
"""Causal multi-head attention (B=2, S=2048, D=2048, 32 heads x 64) for 8
Trainium2 NeuronCores.

Sharding: data parallel on batch (2 groups of 4 cores) x tensor parallel on
heads (4 groups of 8 heads each). Each core computes q/k/v projections for
its head group, RoPE, causal attention with sigmoid-gated values, and a
partial o-projection; the host sums the 4 partials per batch (the
"all-reduce" of the o-projection) and adds the output bias + gate-mean
constant.

Design (evolved from the ~481us bf16 kernel):
- q/k projections and the AV matmul run in fp8 DoubleRow perf mode (two
  128-deep k-tiles contracted per instruction, ~1.5x bf16 throughput at
  512-wide moving operands). x is DMA'd once in bf16 and down-converted
  to fp8 on the idle gpsimd engine; q/k weights are scaled x16 on the
  host so W*16 sits in e4m3's good range. The 1/16 compensation folds
  into the RoPE cos/sin tables; the k-side 1/8 softmax scale folds into
  the exp activation's input scale.
- The V projection stays bf16: v/gate noise feeds the output directly
  (no softmax averaging), and the fp8 v-path alone costs ~1.5e-2 of the
  2e-2 error budget.
- Gates are stored hi-lo in one 128-wide fp8 block per head:
  [64 x tanh-hi | ones | 63 x 16*(tanh - hi)]. The DoubleRow AV matmul
  emits hi rows, the softmax denominator, and a 16x-scaled residual in
  one pass (output partitions 65 -> 128 are otherwise idle PE columns,
  so the residual is free); the normalize combines hi + lo/16, cutting
  gate quantization error ~20x. Dim 63 rides hi-only.
- at2 = exp(s/8 - 2) is written as fp8e5 (e5m2) straight from the
  activation engine: scores reach 8.2, and e5m2's 57344 max makes
  overflow impossible where e4m3's 240 NaN'd; at2 precision cancels in
  the softmax ratio for concentrated rows, so the mantissa loss is
  cheap.
- Scores contract only 64 partitions (head dim) so fp8 buys nothing
  there; they stay bf16. Causal masking is a PE identity-matmul
  accumulation, width-restricted (with scores/exp/AV) to pair-aligned
  windows so fully-dead key-tile columns left of the diagonal are never
  touched.
- o-projection stays bf16 end to end (fp8 y/Wo alone measures 2.8e-2).
- PSUM pools stay open across phases; software-pipelined interleaving
  keeps the PE dense: attention for query blocks 0-1 is woven into the
  half-1 projection stream; o-projs of finished blocks spread through
  the next block's attention; AV matmuls trail their exp by two pairs.
- RoPE uses a host-side d-permutation (pairs (d, d+32) on adjacent
  partitions) so rotate-half is a within-quadrant stream_shuffle lane
  swap; weights, biases and cos/sin tables are permuted to match.
- Gate = tanh(v/2) = 2*sigmoid(v)-1: the 0.5 folds into Wo and the +0.5
  mean term becomes a host constant; tanh shares the exp activation
  table.
"""

import os

import numpy as np
import ml_dtypes

import concourse.bacc as bacc
import concourse.tile as tile
from concourse import mybir
from concourse.bass_utils import run_bass_kernel_spmd

B, S, D = 2, 2048, 2048
H_PER_CORE = 8          # heads per core
DH = 64                 # head dim
CW = 512                # per-core projection width = H_PER_CORE * DH
N_CORES = 8
KT = D // 128           # k-subtiles for the D-contraction
VST = 1024              # va tile-block stride (8 heads x 128)
WS = 16.0               # fp8 weight pre-scale

f32 = mybir.dt.float32
bf16 = mybir.dt.bfloat16
fp8 = mybir.dt.float8e4
fp8e5 = mybir.dt.float8e5
Act = mybir.ActivationFunctionType
Alu = mybir.AluOpType
DR = mybir.MatmulPerfMode.DoubleRow

TRACE = bool(int(os.environ.get("KERNEL_TRACE", "0")))
LAST_EXEC_NS = None
LAST_MEAN_NS = None

_SENT = object()
_SWAP_MASK = [i ^ 1 for i in range(32)]


def _build(WITH_BIAS=True):
    nc = bacc.Bacc("TRN2", target_bir_lowering=False, debug=False)

    x4 = nc.dram_tensor("x4", [4, 128, KT, 512], bf16, kind="ExternalInput")
    wq4 = nc.dram_tensor("wq4", [4, 128, KT, 128], fp8, kind="ExternalInput")
    wk4 = nc.dram_tensor("wk4", [4, 128, KT, 128], fp8, kind="ExternalInput")
    wv4 = nc.dram_tensor("wv4", [128, KT, CW], bf16, kind="ExternalInput")
    wo4 = nc.dram_tensor("wo4", [4, 128, 4, 512], bf16, kind="ExternalInput")
    bq = nc.dram_tensor("bq", [1, CW], bf16, kind="ExternalInput")
    bk = nc.dram_tensor("bk", [1, CW], bf16, kind="ExternalInput")
    bv = nc.dram_tensor("bv", [1, CW], bf16, kind="ExternalInput")
    ropec = nc.dram_tensor("ropec", [4, 128, 512], f32, kind="ExternalInput")
    ropes = nc.dram_tensor("ropes", [4, 128, 512], f32, kind="ExternalInput")
    masks = nc.dram_tensor("masks", [128, 4, 512], bf16, kind="ExternalInput")
    ident = nc.dram_tensor("ident", [128, 128], bf16, kind="ExternalInput")
    vinit = nc.dram_tensor("vinit", [128, 16 * VST], fp8, kind="ExternalInput")
    part = nc.dram_tensor("part", [S, D], bf16, kind="ExternalOutput")

    with tile.TileContext(nc) as tc:
        with (
            tc.tile_pool(name="p0", bufs=1) as p0,
            tc.tile_pool(name="pqk", bufs=1) as pqk,
            tc.tile_pool(name="py", bufs=1) as py,
            tc.tile_pool(name="pa", bufs=1) as pa,
            tc.tile_pool(name="paw", bufs=5) as paw,
            tc.tile_pool(name="prc", bufs=1) as prc,
            tc.tile_pool(name="prt", bufs=2) as prt,
            tc.tile_pool(name="prv", bufs=2) as prv,
            tc.tile_pool(name="pba", bufs=4) as pba,
            tc.tile_pool(name="pbs", bufs=2) as pbs,
            tc.tile_pool(name="pc", bufs=2) as pc,
            tc.tile_pool(name="pbo", bufs=2) as pbo,
        ):
            # persistent state
            qt_all = pqk.tile([128, 4, S], bf16, name="qt_all")
            kt_all = pqk.tile([128, 4, S], bf16, name="kt_all")
            qt = [qt_all[:, i, :] for i in range(4)]
            kt = [kt_all[:, i, :] for i in range(4)]
            # gated values: 16 seq-tiles x 8 heads x [64 hi | ones | 63 lo]
            va_all = p0.tile([128, 16 * VST], fp8, name="va_all")
            va4 = va_all[:].rearrange("p (k h d) -> p k h d", k=16, h=8)
            onesb = p0.tile([1, CW], bf16, name="onesb")
            bvt = p0.tile([1, CW], bf16, name="bvt")
            bqrow = p0.tile([1, CW], bf16, name="bqrow")
            bkrow = p0.tile([1, CW], bf16, name="bkrow")
            idt = p0.tile([128, 128], bf16, name="idt")
            maskt = p0.tile([128, 4, 512], bf16, name="maskt")
            wvf = p0.tile([128, KT, CW], bf16, name="wvf")
            ytr = [py.tile([128, S], bf16, name=f"ytr{i}") for i in range(4)]
            bneg = p0.tile([128, 1], f32, name="bneg")

            warm = p0.tile([1, 8], f32, name="warm")

            def load_xh(half, first_wch=None):
                xhv = pa.tile([128, 2, KT, 512], bf16, tag="xhv", name="xhv")
                xh8 = pa.tile([128, 2, KT, 512], fp8, tag="xh8", name="xh8")
                if first_wch is not None:
                    first_wch()
                for kg in range(4):
                    for qloc in range(2):
                        nc.sync.dma_start(
                            xhv[:, qloc, 4 * kg:4 * kg + 4, :],
                            x4[2 * half + qloc, :, 4 * kg:4 * kg + 4, :])
                        # fp8 shadow for the DoubleRow q/k matmuls. Act
                        # engine: idle during projection phases, and ~5x
                        # faster at dtype casts than the gpsimd DSPs.
                        nc.scalar.activation(
                            xh8[:, qloc, 4 * kg:4 * kg + 4, :],
                            xhv[:, qloc, 4 * kg:4 * kg + 4, :], Act.Copy)
                return xhv, xh8

            def load_rope(half):
                cosw = prc.tile([128, 2, 512], f32, tag="tblc", name="cosw")
                rsnw = prc.tile([128, 2, 512], f32, tag="tbls", name="rsnw")
                for qloc in range(2):
                    nc.sync.dma_start(cosw[:, qloc, :], ropec[2 * half + qloc])
                    nc.sync.dma_start(rsnw[:, qloc, :], ropes[2 * half + qloc])
                return cosw, rsnw

            def emit_consts():
                nc.sync.dma_start(idt[:], ident[:])
                nc.sync.dma_start(maskt[:], masks[:])
                nc.vector.memset(bneg[:], -2.0)
                if WITH_BIAS:
                    nc.vector.memset(onesb[:], 1.0)
                    nc.sync.dma_start(bvt[:], bv[:])
                    nc.sync.dma_start(bqrow[:], bq[:])
                    nc.sync.dma_start(bkrow[:], bk[:])

            def emit_vconsts():
                # needed only from the V-projection phase onward
                for kg in range(4):
                    nc.sync.dma_start(wvf[:, 4 * kg:4 * kg + 4, :],
                                      wv4[:, 4 * kg:4 * kg + 4, :])
                # va default 1.0 -> per-head ones column stays 1 (softmax
                # denominator rides the AV matmul)
                for vg in range(4):
                    nc.sync.dma_start(
                        va_all[:, vg * 4 * VST:(vg + 1) * 4 * VST],
                        vinit[:, vg * 4 * VST:(vg + 1) * 4 * VST])

            def load_wch(w3, mt):
                wch = paw.tile([128, KT, 128], fp8, tag="wch", name="wch")
                nc.sync.dma_start(wch[:], w3[mt])
                return wch

            def qk_unit(wch, dall, brow, mt, half, xh8, cosw, rsnw, pmain,
                        hook=None):
                ps = pmain.tile([128, 2, 512], f32, tag="psa", name="ps_a")
                for kp in range(KT // 2):
                    if kp == 4 and hook is not None:
                        hook()
                    for qloc in range(2):
                        # qloc inner: consecutive matmuls share the
                        # stationary weight pair
                        nc.tensor.matmul(
                            ps[:, qloc, :], wch[:, 2 * kp:2 * kp + 2, :],
                            xh8[:, qloc, 2 * kp:2 * kp + 2, :],
                            start=(kp == 0),
                            stop=(kp == KT // 2 - 1 and not WITH_BIAS),
                            perf_mode=DR,
                        )
                if WITH_BIAS:
                    for qloc in range(2):
                        nc.tensor.matmul(
                            ps[:, qloc, :],
                            brow[0:1, mt * 128:(mt + 1) * 128],
                            onesb[0:1, 0:512], start=False, stop=True,
                        )
                # RoPE with host-interleaved d-order (pairs (d, d+32) sit on
                # adjacent partitions; scores are invariant to the shared
                # q/k permutation): rotate-half becomes a within-quadrant
                # stream_shuffle lane swap, so every op is full-width and
                # partition-aligned. cos/sin tables carry the 1/WS fp8
                # compensation.
                d3 = dall[:, mt, half * 1024:(half + 1) * 1024
                          ].rearrange("p (a b) -> p a b", a=2)
                tmp = prt.tile([128, 2, 512], bf16, tag="tmp", name="tmp")
                tmp2 = prt.tile([128, 2, 512], bf16, tag="tmp2", name="tmp2")
                nc.vector.tensor_mul(tmp[:], ps[:], rsnw[:])
                nc.vector.stream_shuffle(tmp2[:], tmp[:], _SWAP_MASK)
                nc.vector.tensor_mul(d3[:], ps[:], cosw[:])
                nc.vector.tensor_add(d3[:], d3[:], tmp2[:])

            def v_unit(qloc, sp, half, xhv, pmain, hook=None):
                # two seq tiles (st = 2*sp, 2*sp+1) per PSUM pair; bf16
                # matmuls (fp8 v noise feeds the output un-averaged)
                qtr = 2 * half + qloc
                stg0 = qtr * 4 + 2 * sp
                psb = pmain.tile([128, 2, 512], f32, tag="psa", name="ps_v")
                for j in range(2):
                    if j == 1 and hook is not None:
                        hook()
                    st = 2 * sp + j
                    for k in range(KT):
                        nc.tensor.matmul(
                            psb[:, j, :],
                            xhv[:, qloc, k, st * 128:(st + 1) * 128],
                            wvf[:, k, :],
                            start=(k == 0),
                            stop=(k == KT - 1 and not WITH_BIAS),
                        )
                    if WITH_BIAS:
                        nc.tensor.matmul(
                            psb[:, j, :], onesb[0:1, 0:128], bvt[:],
                            start=False, stop=True,
                        )
                # gate = tanh(v/2) = 2*sigmoid(v)-1 (0.5 folded into Wo,
                # +0.5 mean term added on host). Full-precision tanh lands
                # in bf16; the fp8 block gets hi = fp8(g) and a 16x-scaled
                # residual lo (dims 0..62) for the hi-lo AV matmul.
                vab = va_all[:, VST * stg0:VST * (stg0 + 2)].rearrange(
                    "p (s h d) -> p s h d", s=2, h=8)
                tvt = prv.tile([128, 2, 8, 64], bf16, tag="tvt", name="tvt")
                nc.scalar.activation(
                    tvt[:], psb[:].rearrange("p a (h d) -> p a h d", h=8),
                    Act.Tanh, scale=0.5,
                )
                nc.vector.tensor_copy(vab[:, :, :, 0:64], tvt[:])
                # residual stored unscaled: e4m3 subnormals still cut gate
                # quantization ~17x, and the normalize needs no rescale.
                # Layout [64 hi | 63 lo | ones@127] keeps every engine
                # access on a 32-aligned partition base downstream.
                nc.vector.tensor_sub(vab[:, :, :, 64:127],
                                     tvt[:, :, :, 0:63],
                                     vab[:, :, :, 0:63])

            def att_head(qb, pi, hh, psco, psy):
                npair = 2 * qb + 2
                h = 2 * pi + hh
                lo, hi = hh * 64, (hh + 1) * 64
                yps = psy.tile([128, 512], f32, tag="yps", name="ps_y")

                def _w0(kp):
                    # pair-aligned causal window: key pair kp is fully dead
                    # for query columns < 256*(kp - 2*qb)
                    return 0 if kp < 2 * qb else 256 * (kp - 2 * qb)

                def _av(at2, kp):
                    w0 = _w0(kp)
                    nc.tensor.matmul(
                        yps[:, w0:], va4[:, 2 * kp:2 * kp + 2, h, :],
                        at2[:, :, w0:],
                        start=(kp == 0), stop=(kp == npair - 1),
                        perf_mode=DR,
                    )

                pipe = []
                for kp in range(npair):
                    w0 = _w0(kp)
                    ps2 = psco.tile([128, 2, 512], f32, tag="pss",
                                    name="ps_s")
                    for j in range(2):
                        k_i = 2 * kp + j
                        dt_i = k_i - 4 * qb
                        nc.tensor.matmul(
                            ps2[:, j, w0:],
                            kt[pi][lo:hi, k_i * 128:(k_i + 1) * 128],
                            qt[pi][lo:hi, qb * 512 + w0:(qb + 1) * 512],
                            start=True, stop=(dt_i < 0),
                        )
                        if dt_i >= 0:
                            wend = 128 * (dt_i + 1)
                            nc.tensor.matmul(
                                ps2[:, j, w0:wend], idt[:],
                                maskt[:, dt_i, w0:wend],
                                start=False, stop=True,
                            )
                    at2 = pba.tile([128, 2, 512], fp8e5, tag="at", name="at2")
                    # s/8 folds the softmax scale (k weights ship x16, not
                    # x2); e5m2 output cannot overflow (max score ~8.2)
                    nc.scalar.activation(at2[:, :, w0:], ps2[:, :, w0:],
                                         Act.Exp, bias=bneg[:], scale=0.125)
                    if len(pipe) == 2:
                        _av(*pipe.pop(0))
                        yield
                    pipe.append((at2, kp))
                for item in pipe:
                    _av(*item)
                    yield
                # engine partition bases must be 32-aligned and only one
                # TensorTensor input may read PSUM, so: pull quadrant 3 to
                # SBUF, extract the den row (partition 127) with a tiny
                # partition-shifting DMA on the Act queue, then combine
                # hi + lo in SBUF.
                dsb = pbs.tile([32, 512], f32, tag="dsb", name="dsb")
                nc.vector.tensor_copy(dsb[:], yps[96:128, :])
                den = pbs.tile([1, 512], f32, tag="den", name="den")
                nc.gpsimd.dma_start(den[:], dsb[31:32, :])
                rc = pbs.tile([1, 512], f32, tag="rc", name="rc")
                nc.vector.reciprocal_approx_fast(rc[:], den[:])
                s128 = pbs.tile([128, 512], f32, tag="s128", name="s128")
                nc.gpsimd.partition_broadcast(s128[:], rc[:])
                nc.vector.tensor_mul(
                    ytr[pi][lo:hi, qb * 512:(qb + 1) * 512],
                    yps[0:64, :], s128[lo:hi, :],
                )
                # tlo written at base `lo` so the SBUF-SBUF add below has
                # equal base partitions (a TensorTensor requirement)
                tlo = pbs.tile([128, 512], bf16, tag="tlo", name="tlo")
                nc.vector.tensor_mul(tlo[lo:lo + 64, :], yps[64:128, :],
                                     s128[64:128, :])
                nc.vector.tensor_add(
                    ytr[pi][lo:lo + 63, qb * 512:(qb + 1) * 512],
                    ytr[pi][lo:lo + 63, qb * 512:(qb + 1) * 512],
                    tlo[lo:lo + 63, :],
                )

            def att_qbs(qbs, psco, psy):
                for qb in qbs:
                    for pi in range(4):
                        for hh in range(2):
                            yield from att_head(qb, pi, hh, psco, psy)

            def oproj_gen(qb, pso):
                for nt in range(4):
                    woc = pc.tile([128, 4, 512], bf16, tag="woc", name="woc")
                    nc.sync.dma_start(woc[:], wo4[nt])
                    for sl in range(4):
                        st = 4 * qb + sl
                        ps = pso.tile([128, 512], f32, tag="pso", name="ps_o")
                        for kc in range(4):
                            nc.tensor.matmul(
                                ps[:], ytr[kc][:, st * 128:(st + 1) * 128],
                                woc[:, kc, :],
                                start=(kc == 0), stop=(kc == 3),
                            )
                        ostg = pbo.tile([128, 512], bf16, tag="ostg",
                                        name="ostg")
                        # DVE, not Act: staging on the scalar engine queues
                        # behind the exp backlog and strangles o-proj
                        nc.vector.tensor_copy(ostg[:], ps[:])
                        nc.sync.dma_start(
                            part[st * 128:(st + 1) * 128,
                                 nt * 512:(nt + 1) * 512],
                            ostg[:],
                        )
                        yield

            def drive(gen, filler, nf, npair):
                """Pull the attention generator, spreading `nf` filler steps
                proportionally across `npair` attention pairs."""
                n = 0
                pulled = 0
                for _ in gen:
                    n += 1
                    while pulled < n * nf // npair:
                        if next(filler, _SENT) is _SENT:
                            pulled = nf
                            break
                        pulled += 1
                for _ in filler:
                    pass

            wsets = ((wq4, qt_all, bqrow), (wk4, kt_all, bkrow))

            with (
                tc.tile_pool(name="pmain", bufs=2, space="PSUM") as pmain,
                tc.tile_pool(name="psco", bufs=1, space="PSUM") as psco,
                tc.tile_pool(name="psy1", bufs=2, space="PSUM") as psy1,
            ):
                # ------------- half 0: q/k/v projections -------------
                wchq = [None] * 5

                def _first_wch():
                    wchq[0] = load_wch(wq4, 0)

                nc.vector.memset(warm[:], 0.0)
                nc.scalar.activation(warm[:], warm[:], Act.Exp)
                xhv0, xh80 = load_xh(0, _first_wch)
                cosw0, rsnw0 = load_rope(0)
                for i in range(1, 4):
                    wchq[i] = load_wch(wq4, i)
                wchq[4] = load_wch(wk4, 0)
                emit_consts()
                for wi, (w3, dall, brow) in enumerate(wsets):
                    for mt in range(4):
                        pre = 4 * wi + mt
                        wch = wchq[pre] if pre < 5 else load_wch(w3, mt)
                        if pre == 2:
                            emit_vconsts()
                        qk_unit(wch, dall, brow, mt, 0, xh80, cosw0, rsnw0,
                                pmain)
                for qloc in range(2):
                    for sp in range(2):
                        v_unit(qloc, sp, 0, xhv0, pmain)

                # ---- half 1 interleaved with attention qb0/qb1 ----
                xhv1, xh81 = load_xh(1)
                cosw1, rsnw1 = load_rope(1)
                g01 = att_qbs((0, 1), psco, psy1)

                def take(n):
                    for _ in range(n):
                        if next(g01, _SENT) is _SENT:
                            return

                for w3, dall, brow in wsets:
                    for mt in range(4):
                        take(2)
                        qk_unit(load_wch(w3, mt), dall, brow, mt, 1, xh81,
                                cosw1, rsnw1, pmain, hook=lambda: take(2))
                for qloc in range(2):
                    for sp in range(2):
                        take(2)
                        v_unit(qloc, sp, 1, xhv1, pmain,
                               hook=lambda: take(2))
                for _ in g01:
                    pass

            # ---- attention qb2/qb3 with o-proj of finished blocks ----
            with (
                tc.tile_pool(name="pssB", bufs=2, space="PSUM") as pssB,
                tc.tile_pool(name="psyB", bufs=2, space="PSUM") as psyB,
                tc.tile_pool(name="pso", bufs=2, space="PSUM") as pso,
            ):
                import itertools
                f01 = itertools.chain(oproj_gen(0, pso), oproj_gen(1, pso))
                drive(att_qbs((2,), pssB, psyB),
                      itertools.islice(f01, 24), 24, 48)
                drive(att_qbs((3,), pssB, psyB),
                      itertools.chain(f01, oproj_gen(2, pso)), 24, 64)
                for _ in oproj_gen(3, pso):
                    pass

    nc.compile()
    return nc


def _rope_tables():
    half = DH // 2
    inv_freq = 1.0 / (10000.0 ** (np.arange(0, half, dtype=np.float32) / half))
    t = np.arange(S, dtype=np.float32)
    freqs = np.einsum("i,j->ij", t, inv_freq)            # [S, 32]
    emb = np.concatenate([freqs, freqs], axis=-1)        # [S, 64]
    cos = np.cos(emb).T.astype(np.float32)                        # [64, S]
    sin = np.sin(emb).T.astype(np.float32)
    rsin = np.concatenate([-sin[:32], sin[32:]], axis=0)
    return np.ascontiguousarray(np.concatenate([cos, rsin], axis=0))  # [128, S]


def _masks():
    j = np.arange(128)[:, None, None]
    dt = np.arange(4)[None, :, None]
    i = np.arange(512)[None, None, :]
    keep = (128 * dt + j) <= i
    return np.where(keep, 0.0, -1e30).astype(np.float32)  # [128, 4, 512]


def _bf(a):
    return np.ascontiguousarray(a).astype(ml_dtypes.bfloat16)


def _f8(a):
    return np.ascontiguousarray(a).astype(ml_dtypes.float8_e4m3)


def kernel(**inputs):
    global LAST_EXEC_NS
    x = np.asarray(inputs["x"], dtype=np.float32)
    Wq = np.asarray(inputs["Wq"], dtype=np.float32)
    Wk = np.asarray(inputs["Wk"], dtype=np.float32)
    Wv = np.asarray(inputs["Wv"], dtype=np.float32)
    Wo = np.asarray(inputs["Wo"], dtype=np.float32)
    bq = np.asarray(inputs["bq"], dtype=np.float32)
    bk = np.asarray(inputs["bk"], dtype=np.float32)
    bv = np.asarray(inputs["bv"], dtype=np.float32)
    bo = np.asarray(inputs["bo"], dtype=np.float32)

    ropeT = _rope_tables()
    masks = _masks()

    with_bias = any(float(np.abs(b).max()) > 0 for b in (bq, bk, bv))
    nc = _build(WITH_BIAS=with_bias)
    # d-order interleave: RoPE partner (d, d+32) -> adjacent partitions, so
    # rotate-half is a stream_shuffle lane swap. Scores are invariant to the
    # shared q/k permutation; W columns, biases and tables permute together.
    perm64 = np.empty(64, np.int64)
    perm64[0::2] = np.arange(32)          # new 2j   <- old j
    perm64[1::2] = np.arange(32) + 32     # new 2j+1 <- old j+32
    idx512 = (np.arange(0, 512, 64)[:, None] + perm64[None, :]).reshape(512)
    cos64 = ropeT[0:64][perm64]                       # [64, S] permuted
    cosT = np.concatenate([cos64, cos64], axis=0)     # [128, S] dup
    sin0 = -ropeT[64:96]                              # +sin rows (j<32)
    rs64 = np.empty((64, S), np.float32)
    rs64[0::2] = sin0                                 # new 2j   -> +sin_j
    rs64[1::2] = -sin0                                # new 2j+1 -> -sin_j
    rsT = np.concatenate([rs64, rs64], axis=0)        # [128, S]
    # 1/WS compensates the x16 fp8 weight pre-scale
    ropec = np.ascontiguousarray(
        (cosT / WS).reshape(128, 4, 512).transpose(1, 0, 2))
    ropes = np.ascontiguousarray(
        (rsT / WS).reshape(128, 4, 512).transpose(1, 0, 2))
    vinit = _f8(np.ones((128, 16 * VST), dtype=np.float32))
    ident = _bf(np.eye(128, dtype=np.float32))
    masks_b = _bf(masks)
    in_maps = []
    for c in range(N_CORES):
        b, g = c // 4, c % 4
        sl = slice(CW * g, CW * (g + 1))
        xT = x[b].T                                    # [D, S]
        x4 = _bf(xT.reshape(KT, 128, 4, 512).transpose(2, 1, 0, 3))
        wq4 = _f8((Wq[sl][idx512].T * WS)
                  .reshape(KT, 128, 4, 128).transpose(2, 1, 0, 3))
        # k ships x16 like q; the softmax 1/8 folds into the exp scale
        wk4 = _f8((Wk[sl][idx512].T * WS)
                  .reshape(KT, 128, 4, 128).transpose(2, 1, 0, 3))
        wv4 = _bf(Wv[sl].T.reshape(KT, 128, CW).transpose(1, 0, 2))
        wo4 = _bf(
            (0.5 * Wo[:, sl]).T.reshape(4, 128, 4, 512).transpose(2, 1, 0, 3))
        in_maps.append({
            "x4": x4,
            "wq4": wq4,
            "wk4": wk4,
            "wv4": wv4,
            "wo4": wo4,
            "bq": _bf((bq[sl][idx512] * WS).reshape(1, CW)),
            "bk": _bf((bk[sl][idx512] * WS).reshape(1, CW)),
            "bv": _bf(bv[sl].reshape(1, CW)),
            "ropec": ropec,
            "ropes": ropes,
            "vinit": vinit,
            "ident": ident,
            "masks": masks_b,
        })

    kwargs = {}
    if TRACE:
        kwargs = dict(trace=True, trace_cores=list(range(N_CORES)),
                      stitch_traces=False)
        tdir = os.environ.get("KERNEL_TRACE_DIR")
        if tdir:
            os.makedirs(tdir, exist_ok=True)
            kwargs["tmpdir"] = tdir
    global LAST_MEAN_NS
    r = run_bass_kernel_spmd(nc, in_maps, list(range(N_CORES)), **kwargs)
    LAST_EXEC_NS = r.exec_time_ns
    LAST_MEAN_NS = r.mean_exec_time_ns

    # host "all-reduce": sum the 4 head-group partials per batch, add the
    # output bias and the 0.5*rowsum(Wo) term from the centered gate.
    const = bo + 0.5 * Wo.sum(axis=1)
    out = np.empty((B, S, D), dtype=np.float32)
    for b in range(B):
        acc = r.results[4 * b]["part"].astype(np.float32).copy()
        for g in range(1, 4):
            acc += r.results[4 * b + g]["part"]
        out[b] = acc + const
    return out
